# revision 34
# baseline (speedup 1.0000x reference)
"""DecoderRNN (bidirectional-GRU greedy decoder) Trainium2 kernel, 8-core SPMD.

v3 strategy (bf16-resident projection + fp32 top-3 refinement):
  - Vocab-parallel: each core owns a 4000-row slice of w_out, resident in
    SBUF as bf16 [128, 16*8*500] -> no per-step HBM streaming and 1 cyc/col
    PE streaming (vs 4 for fp32).
  - GRU tensor-parallel over H (fp32, batch-major): each core computes a
    128-wide slice of each gate (both dirs); hidden state AllGathered
    (transposed layout) each step.
  - Greedy argmax exactness: bf16 logits carry ~1.3e-3 noise; per group
    (500 cols) each core extracts top-8/partition-row (vector.max), keeps
    top-2 per (stack, group) -> 16 candidates/batch row, merges to top-3,
    gathers those w_out rows in fp32 from HBM, recomputes exact fp32 logits
    on the PE, and the cross-core argmax (AG2) compares refined fp32 values.
  - log_softmax: per-core sum(exp(logits - m_approx)) via ACT accum_out;
    AG2 carries (m_refined, idx, m_approx, s).
  - PE HAM keep-warm: the PE clock-gates to 1.2 GHz after ~3.4us idle;
    tiny matmul<->ACT ping-pong chains (paced by cross-engine latency)
    bridge the gates/AllGather and embed-gather idle windows.

Layouts (per core k, v0 = 4000*k, hidden slice = 128*k):
  wres   [128, 16*8*500] bf16 w_outT: [p, (c*8+ch)*500+f] = w_out[v0+ch*500+f, c*128+p]
  wih/whh[128, 8*768]    gate-sliced GRU weights, transposed; column order
                         per K-chunk: [f_r f_z b_r b_z | f_n b_n] (128 each)
  hT     [128, 8*64]     full hidden transposed: [p, c*64 + dir*32 + b]
  xT     [128, 8*32]     embedded token transposed: [p, c*32 + b]
  logits [128, 1000]     [32*j + b, g*500 + f] = logits[b, v0+(g*4+j)*500+f]
"""

import numpy as np

import concourse.bass as bass
import concourse.bacc as bacc
import concourse.mybir as mybir
import concourse.tile as tile
import concourse.bass_utils as bass_utils
from concourse.masks import make_identity

F32 = mybir.dt.float32
BF16 = mybir.dt.bfloat16
U32 = mybir.dt.uint32
AF = mybir.ActivationFunctionType
ALU = mybir.AluOpType
AX = mybir.AxisListType

B = 32
H = 1024
V = 32000
NC = 8
Vs = V // NC          # 4000 vocab rows per core
Hs = H // NC          # 128 hidden dims per core
KC = 16               # K-chunks of 128 over 2H
NCH = 8               # n-chunks of 500 over Vs
CH = 500              # n-chunk width (one PSUM bank)
GROUPS = 2            # col-tile groups of 4 chunks
NREF = 3              # refined candidates per batch row
W1 = 14               # keep-warm pairs, gates/AG1 window
W2 = 4                # keep-warm pairs, embed-gather window
BIG = 1.0e30


def build_program(T: int):
    nc = bacc.Bacc("TRN2", target_bir_lowering=False, debug=False, num_devices=NC)

    emb_t = nc.dram_tensor("emb_t", [V, H], F32, kind="ExternalInput")
    wres_t = nc.dram_tensor("wres_t", [128, KC * NCH * CH], BF16, kind="ExternalInput")
    wsl_t = [nc.dram_tensor(f"wsl{h}_t", [Vs, 1024], F32, kind="ExternalInput")
             for h in range(2)]
    boutf_t = nc.dram_tensor("boutf_t", [Vs, 1], F32, kind="ExternalInput")
    wih_t = nc.dram_tensor("wih_t", [128, 8 * 768], F32, kind="ExternalInput")
    whh_t = nc.dram_tensor("whh_t", [128, 8 * 768], F32, kind="ExternalInput")
    brz_t = nc.dram_tensor("brz_t", [B, 512], F32, kind="ExternalInput")
    bin_t = nc.dram_tensor("bin_t", [B, 256], F32, kind="ExternalInput")
    bhn_t = nc.dram_tensor("bhn_t", [B, 256], F32, kind="ExternalInput")
    bout_t = nc.dram_tensor("bout_t", [128, GROUPS * CH], BF16, kind="ExternalInput")
    offsl_t = nc.dram_tensor("offsl_t", [128, 1], F32, kind="ExternalInput")
    v0_t = nc.dram_tensor("v0_t", [B, 1], F32, kind="ExternalInput")
    onehot_t = nc.dram_tensor("onehot_t", [128, B], F32, kind="ExternalInput")
    ht0_t = nc.dram_tensor("ht0_t", [128, 8 * 64], F32, kind="ExternalInput")
    hbm0_t = nc.dram_tensor("hbm0_t", [B, 256], F32, kind="ExternalInput")
    x0t_t = nc.dram_tensor("x0t_t", [128, 8 * 32], F32, kind="ExternalInput")
    logp_t = nc.dram_tensor("logp_t", [T * 128, GROUPS * CH], F32, kind="ExternalOutput")

    rg = [list(range(NC))]

    with tile.TileContext(nc) as tc:
        with (
            tc.tile_pool(name="const", bufs=1) as cpool,
            tc.tile_pool(name="gate", bufs=1) as gpool,
            tc.tile_pool(name="lg", bufs=1) as lpool,
            tc.tile_pool(name="ref", bufs=1) as rpool,
            tc.tile_pool(name="stats", bufs=1) as tpool,
            tc.tile_pool(name="ps_rz", bufs=1, space="PSUM") as ps_rz_pool,
            tc.tile_pool(name="ps_n", bufs=1, space="PSUM") as ps_n_pool,
            tc.tile_pool(name="ps_proj", bufs=1, space="PSUM") as ps_proj_pool,
            tc.tile_pool(name="ps_tr", bufs=1, space="PSUM") as ps_tr_pool,
            tc.tile_pool(name="ps_exp", bufs=1, space="PSUM") as ps_exp_pool,
            tc.tile_pool(name="ps_ref", bufs=1, space="PSUM") as ps_ref_pool,
            tc.tile_pool(name="ps_warm", bufs=1, space="PSUM") as ps_warm_pool,
            tc.tile_pool(name="dram", bufs=2, space="DRAM") as dpool,
        ):
            # ---- resident loads ----
            ident = cpool.tile([128, 128], F32, name="ident")
            make_identity(nc, ident[:])
            id32 = ident[0:32, 0:32]
            wres = cpool.tile([128, KC * NCH * CH], BF16, name="wres")
            nc.sync.dma_start(wres[:], wres_t.ap())
            wih = cpool.tile([128, 8 * 768], F32, name="wih")
            nc.sync.dma_start(wih[:], wih_t.ap())
            whh = cpool.tile([128, 8 * 768], F32, name="whh")
            nc.sync.dma_start(whh[:], whh_t.ap())
            brz = cpool.tile([B, 512], F32, name="brz")
            nc.sync.dma_start(brz[:], brz_t.ap())
            b_in = cpool.tile([B, 256], F32, name="b_in")
            nc.sync.dma_start(b_in[:], bin_t.ap())
            b_hn = cpool.tile([B, 256], F32, name="b_hn")
            nc.sync.dma_start(b_hn[:], bhn_t.ap())
            bout = cpool.tile([128, GROUPS * CH], BF16, name="bout")
            nc.sync.dma_start(bout[:], bout_t.ap())
            offsl = cpool.tile([128, 1], F32, name="offsl")
            nc.sync.dma_start(offsl[:], offsl_t.ap())
            v0t = cpool.tile([B, 1], F32, name="v0t")
            nc.sync.dma_start(v0t[:], v0_t.ap())
            onehot = cpool.tile([128, B], F32, name="onehot")
            nc.sync.dma_start(onehot[:], onehot_t.ap())
            bigt = cpool.tile([B, 16], F32, name="bigt")
            nc.vector.memset(bigt[:], BIG)
            warm_c = cpool.tile([32, 16], F32, name="warm_c")
            nc.vector.memset(warm_c[:], 1.0)

            # ping-pong state
            hT = [cpool.tile([128, 8 * 64], F32, name=f"hT{i}") for i in range(2)]
            xT = [cpool.tile([128, 8 * 32], F32, name=f"xT{i}") for i in range(2)]
            hbm = [cpool.tile([B, 256], F32, name=f"hbm{i}") for i in range(2)]
            nc.sync.dma_start(hT[0][:], ht0_t.ap())
            nc.sync.dma_start(xT[0][:], x0t_t.ap())
            nc.sync.dma_start(hbm[0][:], hbm0_t.ap())

            def warm_chain(root_ap, n, label):
                """PE keep-warm: tiny MM -> ACT copy -> tiny MM ..., paced by
                cross-engine latency (~0.5-0.7us per pair). root_ap gates the
                start of the chain (an SBUF f32 AP with >=16 cols)."""
                prev = root_ap
                for i in range(n):
                    wps = ps_warm_pool.tile([16, 16], F32, name="wps", tag="warm")
                    kp = prev.partition_size() if hasattr(prev, "partition_size") else 32
                    nc.tensor.matmul(wps[:], lhsT=prev, rhs=warm_c[0:kp, :],
                                     start=True, stop=True, skip_group_check=True)
                    wsb = gpool.tile([16, 16], F32, name=f"wsb_{label}",
                                     tag=f"wsb_{label}_{i % 2}")
                    nc.scalar.activation(wsb[:], wps[:], AF.Copy)
                    prev = wsb[0:16, 0:16]

            def emit_gh(t, rz_ps, inhn_ps):
                """h-side GRU matmuls for step t (reads hT[t%2] = h(t-1))."""
                h = hT[t % 2]
                hn_ps = inhn_ps[:, 0:256]
                for c in range(8):
                    hf = h[:, c * 64 : c * 64 + 32]
                    hb = h[:, c * 64 + 32 : c * 64 + 64]
                    w = whh[:, c * 768 : (c + 1) * 768]
                    nc.tensor.matmul(rz_ps[:, 0:256], lhsT=hf, rhs=w[:, 0:256],
                                     start=(c == 0), stop=False)
                    nc.tensor.matmul(rz_ps[:, 256:512], lhsT=hb, rhs=w[:, 256:512],
                                     start=False, stop=False)
                    nc.tensor.matmul(hn_ps[:, 0:128], lhsT=hf, rhs=w[:, 512:640],
                                     start=(c == 0), stop=False)
                    nc.tensor.matmul(hn_ps[:, 128:256], lhsT=hb, rhs=w[:, 640:768],
                                     start=False, stop=False)

            # step-0 h-side prologue
            rz_ps_next = ps_rz_pool.tile([B, 512], F32, name="rz_ps", tag="rz")
            inhn_ps_next = ps_n_pool.tile([B, 512], F32, name="inhn_ps", tag="inhn")
            emit_gh(0, rz_ps_next, inhn_ps_next)

            for t in range(T):
                rz_ps = rz_ps_next
                inhn_ps = inhn_ps_next
                x = xT[t % 2]
                h_prev = hbm[t % 2]
                h_cur = hT[(t + 1) % 2]   # written by AG1(t)

                hn_ps = inhn_ps[:, 0:256]
                in_ps = inhn_ps[:, 256:512]
                # ---- x-side GRU matmuls ----
                for c in range(8):
                    xc = x[:, c * 32 : (c + 1) * 32]
                    w = wih[:, c * 768 : (c + 1) * 768]
                    nc.tensor.matmul(rz_ps[:], lhsT=xc, rhs=w[:, 0:512],
                                     start=False, stop=(c == 7))
                    nc.tensor.matmul(in_ps, lhsT=xc, rhs=w[:, 512:768],
                                     start=False, stop=(c == 7))

                # ---- gates (batch-major; col order [f_r f_z b_r b_z]) ----
                s_rz = gpool.tile([B, 512], F32, name="s_rz", tag="s_rz")
                nc.vector.tensor_add(s_rz[:], rz_ps[:], brz[:])
                nc.scalar.activation(s_rz[:], s_rz[:], AF.Tanh, scale=0.5)
                nc.vector.tensor_scalar(s_rz[:], s_rz[:], 0.5, 0.5,
                                        op0=ALU.mult, op1=ALU.add)
                i_n = gpool.tile([B, 256], F32, name="i_n", tag="i_n")
                nc.vector.tensor_add(i_n[:], in_ps, b_in[:])
                h_n = gpool.tile([B, 256], F32, name="h_n", tag="h_n")
                nc.vector.tensor_add(h_n[:], hn_ps, b_hn[:])
                # h_n *= r ; h_n += i_n ; n = tanh(h_n)
                nc.vector.tensor_tensor(h_n[:, 0:128], s_rz[:, 0:128],
                                        h_n[:, 0:128], op=ALU.mult)
                nc.vector.tensor_tensor(h_n[:, 128:256], s_rz[:, 256:384],
                                        h_n[:, 128:256], op=ALU.mult)
                nc.vector.tensor_add(h_n[:], h_n[:], i_n[:])
                nc.scalar.activation(h_n[:], h_n[:], AF.Tanh)
                # d = (h_prev - n) * z ; h_new = n + d   (d reuses i_n)
                nc.vector.tensor_sub(i_n[:], h_prev[:], h_n[:])
                nc.vector.tensor_tensor(i_n[:, 0:128], s_rz[:, 128:256],
                                        i_n[:, 0:128], op=ALU.mult)
                nc.vector.tensor_tensor(i_n[:, 128:256], s_rz[:, 384:512],
                                        i_n[:, 128:256], op=ALU.mult)
                h_new = hbm[(t + 1) % 2]
                nc.vector.tensor_add(h_new[:], h_n[:], i_n[:])

                # ---- transpose h_new, AllGather hidden ----
                tr_ps = ps_tr_pool.tile([128, 512], F32, name="tr_ps", tag="tr")
                nc.tensor.matmul(tr_ps[:, 0:32], lhsT=h_new[:, 0:128], rhs=id32,
                                 is_transpose=True, start=True, stop=False)
                nc.tensor.matmul(tr_ps[:, 32:64], lhsT=h_new[:, 128:256], rhs=id32,
                                 is_transpose=True, start=False, stop=True)
                ag1_sb = tpool.tile([128, 64], F32, name="ag1_sb", tag="ag1_sb")
                nc.vector.tensor_copy(ag1_sb[:], tr_ps[:, 0:64])
                ag1_in = dpool.tile([128, 64], F32, name="ag1_in", tag="ag1_in")
                nc.sync.dma_start(ag1_in[:], ag1_sb[:])
                ag1_out = dpool.tile([128 * NC, 64], F32, name="ag1_out",
                                     addr_space="Shared", tag="ag1_out")
                nc.gpsimd.collective_compute(
                    "AllGather", ALU.bypass, replica_groups=rg,
                    ins=[ag1_in.opt()], outs=[ag1_out.opt()])
                # keep-warm through gates/AG1/unpack window
                warm_chain(h_new[:, 0:16], W1, "a")
                nc.sync.dma_start(
                    h_cur[:].rearrange("p (c q) -> p c q", c=8),
                    ag1_out[:].rearrange("(c p) q -> p c q", p=128))
                # bf16 copy of the gathered hidden for the projection
                hTb = gpool.tile([128, 8 * 64], BF16, name="hTb", tag="hTb")
                nc.vector.tensor_copy(hTb[:], h_cur[:])

                # ---- output projection (bf16), group-major ----
                pj = [ps_proj_pool.tile([128, 512], F32, name=f"pj{g}", tag=f"pj{g}")
                      for g in range(GROUPS)]

                def lh_of(c):
                    if c < 8:
                        return hTb[:, c * 64 : c * 64 + 32]
                    return hTb[:, (c - 8) * 64 + 32 : (c - 8) * 64 + 64]

                logits = lpool.tile([128, GROUPS * CH], F32, name="logits", tag="logits")
                mx8 = [None, None]
                ix8 = [None, None]
                for g in range(GROUPS):
                    for c in range(KC):
                        for j in range(4):
                            ch = g * 4 + j
                            nc.tensor.matmul(
                                pj[g][32 * j : 32 * (j + 1), 0:CH], lhsT=lh_of(c),
                                rhs=wres[:, (c * NCH + ch) * CH : (c * NCH + ch + 1) * CH],
                                start=(c == 0), stop=(c == KC - 1),
                                skip_group_check=True, tile_position=(0, 32 * j))
                    # per-group epilogue overlaps the next group's matmuls
                    lg = logits[:, g * CH : (g + 1) * CH]
                    nc.vector.tensor_add(lg, pj[g][:, 0:CH],
                                         bout[:, g * CH : (g + 1) * CH])
                    mx8[g] = tpool.tile([128, 8], F32, name=f"mx8_{g}", tag=f"mx8_{g}")
                    ix8[g] = tpool.tile([128, 8], U32, name=f"ix8_{g}", tag=f"ix8_{g}")
                    nc.vector.max(out=mx8[g][:], in_=lg)
                    nc.vector.max_index(out=ix8[g][:], in_max=mx8[g][:], in_values=lg)

                # ---- merge: top-3 of 16 candidates (2 per stack x group) ----
                cand16 = tpool.tile([B, 16], F32, name="cand16", tag="cand16")
                candi16 = tpool.tile([B, 16], F32, name="candi16", tag="candi16")
                for g in range(GROUPS):
                    ixf = tpool.tile([128, 2], F32, name=f"ixf{g}", tag=f"ixf{g}")
                    nc.vector.tensor_copy(ixf[:], ix8[g][:, 0:2])
                    # local vocab idx = j*500 + g*2000 + col
                    nc.vector.tensor_scalar(ixf[:], ixf[:], offsl[:, 0:1],
                                            float(g * 2000), op0=ALU.add, op1=ALU.add)
                    for j in range(4):
                        nc.vector.tensor_copy(
                            cand16[:, g * 8 + j * 2 : g * 8 + j * 2 + 2],
                            mx8[g][32 * j : 32 * (j + 1), 0:2])
                        nc.vector.tensor_copy(
                            candi16[:, g * 8 + j * 2 : g * 8 + j * 2 + 2],
                            ixf[32 * j : 32 * (j + 1), 0:2])
                m16 = tpool.tile([B, 8], F32, name="m16", tag="m16")
                nc.vector.max(out=m16[:], in_=cand16[:])
                iloc = tpool.tile([B, NREF], F32, name="iloc", tag="iloc")
                for r in range(NREF):
                    mskr = tpool.tile([B, 16], U32, name="mskr", tag="mskr")
                    nc.vector.tensor_scalar(mskr[:], cand16[:], m16[:, r : r + 1],
                                            None, op0=ALU.is_equal)
                    iselr = tpool.tile([B, 16], F32, name="iselr", tag="iselr")
                    nc.vector.tensor_copy(iselr[:], bigt[:])
                    nc.vector.copy_predicated(iselr[:], mskr[:], candi16[:])
                    nc.vector.tensor_reduce(iloc[:, r : r + 1], iselr[:],
                                            axis=AX.X, op=ALU.min)

                # ---- fp32 refinement of the NREF candidates ----
                # index prep + gathers first (gpsimd/DMA) so the sum-exp pass
                # below overlaps the gather flight
                NP = NREF * 32
                i128f = rpool.tile([128, 1], F32, name="i128f", tag="i128f")
                for r in range(NREF):
                    nc.vector.tensor_copy(i128f[32 * r : 32 * (r + 1), :],
                                          iloc[:, r : r + 1])
                i128 = rpool.tile([128, 1], U32, name="i128", tag="i128")
                nc.vector.tensor_copy(i128[0:NP, :], i128f[0:NP, :])
                bcand = rpool.tile([128, 1], F32, name="bcand", tag="bcand")
                nc.gpsimd.indirect_dma_start(
                    out=bcand[0:NP, :], out_offset=None, in_=boutf_t.ap(),
                    in_offset=bass.IndirectOffsetOnAxis(ap=i128[0:NP, 0:1], axis=0))


                # ---- local sum-exp (vs approx max) — off critical path ----
                mneg_l = tpool.tile([128, 1], F32, name="mneg_l", tag="mneg_l")
                nc.vector.tensor_scalar_mul(mneg_l[0:B, :], m16[:, 0:1], -1.0)
                nc.vector.tensor_copy(mneg_l[B : 2 * B, :], mneg_l[0:B, :])
                nc.vector.tensor_copy(mneg_l[2 * B :, :], mneg_l[0 : 2 * B, :])
                sparts = tpool.tile([128, 2], F32, name="sparts", tag="sparts")
                for g in range(GROUPS):
                    e_ps = ps_exp_pool.tile([128, 512], F32, name="e_ps", tag="exp")
                    nc.scalar.activation(e_ps[:, 0:CH], logits[:, g * CH : (g + 1) * CH],
                                         AF.Exp, bias=mneg_l[:, 0:1],
                                         accum_out=sparts[:, g : g + 1])
                s128 = tpool.tile([128, 1], F32, name="s128", tag="s128")
                nc.vector.tensor_add(s128[:], sparts[:, 0:1], sparts[:, 1:2])
                scand = tpool.tile([B, 4], F32, name="scand", tag="scand")
                for j in range(4):
                    nc.vector.tensor_copy(scand[:, j : j + 1],
                                          s128[32 * j : 32 * (j + 1), :])
                s_loc = tpool.tile([B, 1], F32, name="s_loc", tag="s_loc")
                nc.vector.reduce_sum(s_loc[:], scand[:], axis=AX.X)

                ref_ps = ps_ref_pool.tile([128, 32], F32, name="ref_ps", tag="ref")
                for half in range(2):
                    wcand = rpool.tile([128, 1024], F32, name="wcand", tag="wcand")
                    nc.gpsimd.indirect_dma_start(
                        out=wcand[0:NP, :], out_offset=None,
                        in_=wsl_t[half].ap(),
                        in_offset=bass.IndirectOffsetOnAxis(ap=i128[0:NP, 0:1], axis=0))
                    # transpose 8 blocks in place (2 PSUM fills of 4)
                    for q in range(2):
                        trw_ps = ps_tr_pool.tile([128, 512], F32, name="trw_ps", tag="tr")
                        for mm in range(4):
                            blk = q * 4 + mm
                            nc.tensor.matmul(
                                trw_ps[:, mm * 128 : mm * 128 + NP],
                                lhsT=wcand[0:NP, blk * 128 : (blk + 1) * 128],
                                rhs=ident[0:NP, 0:NP], is_transpose=True,
                                start=(mm == 0), stop=(mm == 3))
                        nc.vector.tensor_copy(
                            wcand[:, q * 512 : (q + 1) * 512], trw_ps[:])
                    for blk in range(8):
                        m = half * 8 + blk
                        nc.tensor.matmul(
                            ref_ps[0:NP, :],
                            lhsT=wcand[:, blk * 128 : blk * 128 + NP],
                            rhs=h_cur[:, blk * 64 + half * 32 : blk * 64 + half * 32 + 32],
                            start=(m == 0), stop=(m == KC - 1))
                # diag extract: refined[p] = ref_ps[p, p % 32] + b_out[cand]
                refd = rpool.tile([128, 32], F32, name="refd", tag="refd")
                nc.vector.tensor_tensor(refd[0:NP, :], ref_ps[0:NP, :],
                                        onehot[0:NP, :], op=ALU.mult)
                refv = rpool.tile([128, 1], F32, name="refv", tag="refv")
                nc.vector.reduce_sum(refv[0:NP, :], refd[0:NP, :], axis=AX.X)
                nc.vector.tensor_add(refv[0:NP, :], refv[0:NP, :], bcand[0:NP, :])
                refc = tpool.tile([B, NREF], F32, name="refc", tag="refc")
                iglob = tpool.tile([B, NREF], F32, name="iglob", tag="iglob")
                for r in range(NREF):
                    nc.vector.tensor_copy(refc[:, r : r + 1],
                                          refv[32 * r : 32 * (r + 1), :])
                nc.vector.tensor_scalar(iglob[:], iloc[:], v0t[:, 0:1], None,
                                        op0=ALU.add)
                m_loc = tpool.tile([B, 1], F32, name="m_loc", tag="m_loc")
                nc.vector.reduce_max(m_loc[:], refc[:], axis=AX.X)
                msk = tpool.tile([B, NREF], U32, name="msk", tag="msk")
                nc.vector.tensor_scalar(msk[:], refc[:], m_loc[:], None, op0=ALU.is_equal)
                isel = tpool.tile([B, NREF], F32, name="isel", tag="isel")
                nc.vector.tensor_copy(isel[:], bigt[:, 0:NREF])
                nc.vector.copy_predicated(isel[:], msk[:], iglob[:])
                i_loc = tpool.tile([B, 1], F32, name="i_loc", tag="i_loc")
                nc.vector.tensor_reduce(i_loc[:], isel[:], axis=AX.X, op=ALU.min)

                # ---- AG2: (m_ref, idx, m_approx, s) from all cores ----
                ag2_sb = tpool.tile([B, 4], F32, name="ag2_sb", tag="ag2_sb")
                nc.vector.tensor_copy(ag2_sb[:, 0:1], m_loc[:])
                nc.vector.tensor_copy(ag2_sb[:, 1:2], i_loc[:])
                nc.vector.tensor_copy(ag2_sb[:, 2:3], m16[:, 0:1])
                nc.vector.tensor_copy(ag2_sb[:, 3:4], s_loc[:])
                ag2_in = dpool.tile([B, 4], F32, name="ag2_in", tag="ag2_in")
                nc.sync.dma_start(ag2_in[:], ag2_sb[:])
                ag2_out = dpool.tile([B * NC, 4], F32, name="ag2_out",
                                     addr_space="Shared", tag="ag2_out")
                nc.gpsimd.collective_compute(
                    "AllGather", ALU.bypass, replica_groups=rg,
                    ins=[ag2_in.opt()], outs=[ag2_out.opt()])
                unp2 = tpool.tile([B, 32], F32, name="unp2", tag="unp2")
                nc.sync.dma_start(
                    unp2[:].rearrange("b (r c) -> b r c", r=NC),
                    ag2_out[:].rearrange("(r b) c -> b r c", b=B))
                vals = bass.AP(unp2.tensor, unp2[:].offset,
                               [unp2[:].ap[0], [4, 8]])
                idxs = bass.AP(unp2.tensor, unp2[:].offset + 1,
                               [unp2[:].ap[0], [4, 8]])
                mtils = bass.AP(unp2.tensor, unp2[:].offset + 2,
                                [unp2[:].ap[0], [4, 8]])
                svals = bass.AP(unp2.tensor, unp2[:].offset + 3,
                                [unp2[:].ap[0], [4, 8]])
                m_glob = tpool.tile([B, 1], F32, name="m_glob", tag="m_glob")
                nc.vector.reduce_max(m_glob[:], vals, axis=AX.X)
                msk2 = tpool.tile([B, 8], U32, name="msk2", tag="msk2")
                nc.vector.tensor_scalar(msk2[:], vals, m_glob[:], None, op0=ALU.is_equal)
                isel2 = tpool.tile([B, 8], F32, name="isel2", tag="isel2")
                nc.vector.tensor_copy(isel2[:], bigt[:, 0:8])
                nc.vector.copy_predicated(isel2[:], msk2[:], idxs)
                i_glob = tpool.tile([B, 1], F32, name="i_glob", tag="i_glob")
                nc.vector.tensor_reduce(i_glob[:], isel2[:], axis=AX.X, op=ALU.min)
                # logZ = max(m_approx) + ln(sum s_k exp(m_approx_k - max))
                mz = tpool.tile([B, 1], F32, name="mz", tag="mz")
                nc.vector.reduce_max(mz[:], mtils, axis=AX.X)
                dmx = tpool.tile([B, 8], F32, name="dmx", tag="dmx")
                nc.vector.tensor_scalar(dmx[:], mtils, mz[:], None, op0=ALU.subtract)
                nc.scalar.activation(dmx[:], dmx[:], AF.Exp)
                nc.vector.tensor_tensor(dmx[:], dmx[:], svals, op=ALU.mult)
                s_glob = tpool.tile([B, 1], F32, name="s_glob", tag="s_glob")
                nc.vector.reduce_sum(s_glob[:], dmx[:], axis=AX.X)
                lns = tpool.tile([B, 1], F32, name="lns", tag="lns")
                nc.scalar.activation(lns[:], s_glob[:], AF.Ln)
                logz = tpool.tile([128, 1], F32, name="logz", tag="logz", bufs=2)
                nc.vector.tensor_add(logz[0:B, :], lns[:], mz[:])
                nc.vector.tensor_copy(logz[B : 2 * B, :], logz[0:B, :])
                nc.vector.tensor_copy(logz[2 * B :, :], logz[0 : 2 * B, :])

                # ---- prefetch for t+1: gh matmuls, token embed, transpose ----
                if t + 1 < T:
                    rz_ps_next = ps_rz_pool.tile([B, 512], F32, name="rz_ps", tag="rz")
                    inhn_ps_next = ps_n_pool.tile([B, 512], F32, name="inhn_ps", tag="inhn")
                    emit_gh(t + 1, rz_ps_next, inhn_ps_next)
                    tok = tpool.tile([B, 1], U32, name="tok", tag="tok")
                    nc.vector.tensor_copy(tok[:], i_glob[:])
                    x_sb = tpool.tile([B, H], F32, name="x_sb", tag="x_sb", bufs=1)
                    nc.gpsimd.indirect_dma_start(
                        out=x_sb[:], out_offset=None, in_=emb_t.ap(),
                        in_offset=bass.IndirectOffsetOnAxis(ap=tok[:, 0:1], axis=0))
                    xtr_ps = ps_tr_pool.tile([128, 512], F32, name="xtr_ps", tag="tr")
                    for c in range(8):
                        nc.tensor.matmul(xtr_ps[:, c * 32 : (c + 1) * 32],
                                         lhsT=x_sb[:, c * 128 : (c + 1) * 128],
                                         rhs=id32, is_transpose=True,
                                         start=(c == 0), stop=(c == 7))
                    nc.vector.tensor_copy(xT[(t + 1) % 2][:], xtr_ps[:, 0:256])

                # ---- logp = logits - logZ; write out ----
                nc.gpsimd.tensor_scalar(logits[:], logits[:], logz[:, 0:1], None,
                                        op0=ALU.subtract)
                nc.sync.dma_start(logp_t.ap()[t * 128 : (t + 1) * 128, :], logits[:])

    nc.compile()
    return nc


def prep_inputs(inputs, hidden, emb, w_ih_f, w_hh_f, b_ih_f, b_hh_f,
                w_ih_b, w_hh_b, b_ih_b, b_hh_b, w_out, b_out):
    """Build the per-core input maps (all numpy, host-side sharding)."""
    BF16_NP = mybir.dt.np(mybir.dt.bfloat16)
    emb = np.ascontiguousarray(np.asarray(emb), dtype=np.float32)
    w_out = np.asarray(w_out)
    tok0 = np.asarray(inputs)[:, 0].astype(np.int64)
    x0 = emb[tok0]                                              # (B, H)
    hidden = np.asarray(hidden)
    h_f0, h_b0 = hidden[0], hidden[1]                           # (B, H)

    x0t = np.ascontiguousarray(x0.T).reshape(8, 128, B).transpose(1, 0, 2) \
        .reshape(128, 8 * B).astype(np.float32)
    ht0 = np.empty((128, 8, 64), dtype=np.float32)
    ht0[:, :, 0:32] = np.ascontiguousarray(h_f0.T).reshape(8, 128, B).transpose(1, 0, 2)
    ht0[:, :, 32:64] = np.ascontiguousarray(h_b0.T).reshape(8, 128, B).transpose(1, 0, 2)
    ht0 = ht0.reshape(128, 8 * 64)

    wihf, whhf = np.asarray(w_ih_f), np.asarray(w_hh_f)
    wihb, whhb = np.asarray(w_ih_b), np.asarray(w_hh_b)
    bihf, bhhf = np.asarray(b_ih_f), np.asarray(b_hh_f)
    bihb, bhhb = np.asarray(b_ih_b), np.asarray(b_hh_b)

    onehot = np.zeros((128, B), dtype=np.float32)
    for p in range(128):
        onehot[p, p % 32] = 1.0

    in_maps = []
    for k in range(NC):
        v0 = Vs * k
        sl = [slice(g * H + Hs * k, g * H + Hs * (k + 1)) for g in range(3)]

        w_slice = np.ascontiguousarray(w_out[v0 : v0 + Vs, :], dtype=np.float32)
        w_oT = np.ascontiguousarray(w_slice.T)                  # (2048, Vs)
        wres = w_oT.reshape(KC, 128, Vs).transpose(1, 0, 2) \
            .reshape(128, KC * Vs).astype(BF16_NP).copy()

        def gcat(wf, wb):
            cols = [wf[sl[0]].T, wf[sl[1]].T, wb[sl[0]].T, wb[sl[1]].T,
                    wf[sl[2]].T, wb[sl[2]].T]
            cat = np.concatenate(cols, axis=1)                   # (1024, 768)
            return cat.reshape(8, 128, 768).transpose(1, 0, 2) \
                .reshape(128, 8 * 768).astype(np.float32).copy()

        def bcast(v):
            return np.broadcast_to(v.astype(np.float32), (B, v.size)).copy()

        brz_ = bcast(np.concatenate([bihf[sl[0]] + bhhf[sl[0]],
                                     bihf[sl[1]] + bhhf[sl[1]],
                                     bihb[sl[0]] + bhhb[sl[0]],
                                     bihb[sl[1]] + bhhb[sl[1]]]))
        b_in_ = bcast(np.concatenate([bihf[sl[2]], bihb[sl[2]]]))
        b_hn_ = bcast(np.concatenate([bhhf[sl[2]], bhhb[sl[2]]]))

        bo_slice = np.asarray(b_out)[v0 : v0 + Vs].astype(np.float32)
        bo = bo_slice.reshape(GROUPS, 4, CH)
        boutt = np.empty((128, GROUPS * CH), dtype=BF16_NP)
        for g in range(GROUPS):
            for j in range(4):
                boutt[32 * j : 32 * (j + 1), g * CH : (g + 1) * CH] = bo[g, j]

        # per-partition (32j+b) local vocab base: j*500
        of = np.empty((128, 1), dtype=np.float32)
        for j in range(4):
            of[32 * j : 32 * (j + 1), 0] = j * CH

        hbm0 = np.concatenate([h_f0[:, Hs * k : Hs * (k + 1)],
                               h_b0[:, Hs * k : Hs * (k + 1)]], axis=1) \
            .astype(np.float32).copy()

        in_maps.append({
            "emb_t": emb, "wres_t": wres,
            **{f"wsl{q}_t": np.ascontiguousarray(w_slice[:, q * 1024 : (q + 1) * 1024])
               for q in range(2)},
            "boutf_t": bo_slice.reshape(Vs, 1).copy(),
            "wih_t": gcat(wihf, wihb), "whh_t": gcat(whhf, whhb),
            "brz_t": brz_, "bin_t": b_in_, "bhn_t": b_hn_,
            "bout_t": boutt, "offsl_t": of,
            "v0_t": np.full((B, 1), float(v0), dtype=np.float32),
            "onehot_t": onehot,
            "ht0_t": ht0, "hbm0_t": hbm0, "x0t_t": x0t,
        })
    return in_maps


_CACHE = {}


def _get_program(T, **kw):
    key = (T, tuple(sorted(kw.items())))
    if key not in _CACHE:
        _CACHE[key] = build_program(T, **kw)
    return _CACHE[key]


def run(T, in_maps, trace=False):
    nc = _get_program(T)
    res = bass_utils.run_bass_kernel_spmd(
        nc, in_maps, core_ids=list(range(NC)), trace=trace)
    outs = []
    for k in range(NC):
        arr = res.results[k]["logp_t"].reshape(T, 4, B, GROUPS, CH)
        outs.append(arr.transpose(2, 0, 3, 1, 4).reshape(B, T, Vs))
    return np.concatenate(outs, axis=2), res


def kernel(inputs, hidden, emb, w_ih_f, w_hh_f, b_ih_f, b_hh_f,
           w_ih_b, w_hh_b, b_ih_b, b_hh_b, w_out, b_out, output_len):
    T = int(output_len)
    in_maps = prep_inputs(inputs, hidden, emb, w_ih_f, w_hh_f, b_ih_f, b_hh_f,
                          w_ih_b, w_hh_b, b_ih_b, b_hh_b, w_out, b_out)
    out, _ = run(T, in_maps)
    return out
